# revision 22
# baseline (speedup 1.0000x reference)
"""LSG (local-sparse-global) block attention on 8 trn2 NeuronCores.

Shape/config facts hardcoded from the problem spec:
  n=2 h=12 t=4096 d=64, BLOCK=128, SPARSE_BLOCK=128, SF=4 -> ts=1024, g=64.
Per 128-token query block b the key set is:
  global (64) | sparse W1 [32b-160,32b-32) | sparse W2 [32b+64,32b+192)
  | local [128(b-1), 128(b+2))                              -> 704 real keys.

Sharding: n*h = 24 pairs, 3 per core (data parallel, no cross-core comm).

Device-side design (per pair, 32 blocks, processed as 16 2-block batches):
 - Host ships Q^T / K^T (d-major, 64 rows duplicated to 128 partitions so
   consecutive matmuls can run concurrently in the two PE row-groups, and
   DMA uses all SBUF ports).
 - Masking is folded into the value side: softmax(s+m) = exp(s)*exp(m)
   normalized, so the host scales each key's value row and the appended
   ones-column by exp(mask) (zero at structural pads). Zero-padded keys
   then produce exp(0)=1 junk probs that contribute exactly 0.
 - scoresT (keys on partitions, queries free) per batch: 5 chunks x 128
   keys per block via K=64 matmuls, row-group alternating; the global
   scores of the two blocks share ONE [128,128] tile (partitions 0:64 =
   block A's 64 global keys, 64:128 = block B's) via column-group
   targeting, trimming the exp free size 1536 -> 1408.
 - One Exp ACT per batch over the whole [128,1408] PSUM tile (bf16 out).
 - ctx matmuls: stationary = probsT chunk (128x128 bf16, FWL), moving =
   (values | exp(mask)) chunk (128, 65). One PSUM group per batch
   accumulates ctx (q, d) plus the softmax denominator in column 64.
 - Normalize on DVE into a per-pair SBUF accumulator [128, 32, 64];
   ONE contiguous 1 MiB output DMA per pair (8 KiB per partition) instead
   of per-batch scattered 256 B-descriptor stores; host untransposes.
"""

import numpy as np
import ml_dtypes
from contextlib import ExitStack

import concourse.bacc as bacc
import concourse.bass as bass
import concourse.tile as tile
from concourse import mybir
from concourse.bass_utils import run_bass_kernel_spmd
from concourse.tile import add_dep_helper

N, H, T, D = 2, 12, 4096, 64
TS, G = 1024, 64
NCORES = 8
PAIRS = (N * H) // NCORES  # 3
NBLK = T // 128            # 32
BF16 = ml_dtypes.bfloat16

# free-axis offsets (elements) within the per-batch score tile [128, 1408]
SEG = 640            # per-block segment: 3 local + W1 + W2 chunks
GOFF = 2 * SEG       # shared global tile at 1280..1408
FTOT = GOFF + 128    # 1408
FPAD = 1536          # ps tile padded to exactly 3 PSUM banks so the two
                     # pool buffers never share a bank with the ACT reader

LAST_RESULTS = None  # BassKernelResults of the most recent run (for test.py)


def build_program(pairs=PAIRS):
    dt = mybir.dt
    nc = bacc.Bacc("TRN2", target_bir_lowering=False, debug=False)

    qtb = nc.dram_tensor("qtb", [pairs, 128, T], dt.bfloat16, kind="ExternalInput").ap()
    ktb = nc.dram_tensor("ktb", [pairs, 128, T + 256], dt.bfloat16, kind="ExternalInput").ap()
    stb = nc.dram_tensor("stb", [pairs, 128, TS + 320], dt.bfloat16, kind="ExternalInput").ap()
    gtb = nc.dram_tensor("gtb", [pairs, 128, 128], dt.bfloat16, kind="ExternalInput").ap()
    vlb = nc.dram_tensor("vlb", [pairs, 128, 34, D + 1], dt.bfloat16, kind="ExternalInput").ap()
    svb = nc.dram_tensor("svb", [pairs, 128, 10, 4, D + 1], dt.bfloat16, kind="ExternalInput").ap()
    gvb = nc.dram_tensor("gvb", [pairs, 128, 2, D + 1], dt.bfloat16, kind="ExternalInput").ap()
    # output in partition-major layout: oT[p, q, blk, :] = ctx[blk*128+q, :]
    oT = nc.dram_tensor("oT", [pairs, 128, NBLK, D], dt.float32, kind="ExternalOutput").ap()

    with tile.TileContext(nc) as tc, ExitStack() as ctx:
        big = ctx.enter_context(tc.tile_pool(name="big", bufs=2))
        probs = ctx.enter_context(tc.tile_pool(name="probs", bufs=3))
        outp = ctx.enter_context(tc.tile_pool(name="outp", bufs=2))
        ps_pool = ctx.enter_context(tc.tile_pool(name="psp", bufs=2, space="PSUM"))
        cx_pool = ctx.enter_context(tc.tile_pool(name="cxp", bufs=2, space="PSUM"))

        # warm-up activation at t=0: pulls the ~2.7 us exp ACT_TABLE_LOAD off
        # the critical path so it overlaps the first input DMAs.
        warm = probs.tile([128, 2], dt.bfloat16, tag="warm")
        nc.scalar.memzero(warm)
        nc.scalar.activation(warm, warm, mybir.ActivationFunctionType.Exp)

        for p in range(pairs):
            # single HWDGE ring: FIFO issue order doubles as transfer priority
            # (SDMA round-robin has no usable QoS). Order = what batch 0 needs,
            # then value tensors, then the bulk.
            eng = nc.sync
            qt = big.tile([128, T], dt.bfloat16, tag="qt")
            kt = big.tile([128, T + 256], dt.bfloat16, tag="kt")
            st = big.tile([128, TS + 320], dt.bfloat16, tag="st")
            gt = big.tile([128, 128], dt.bfloat16, tag="gt")
            vl = big.tile([128, 34, D + 1], dt.bfloat16, tag="vl")
            sv = big.tile([128, 10, 4, D + 1], dt.bfloat16, tag="sv")
            gv = big.tile([128, 2, D + 1], dt.bfloat16, tag="gv")
            if p == 0:
                # split the score-side loads so the first batches' slices land
                # early and the exp stream starts immediately.
                # boundaries are window-aligned so no batch's slice straddles
                # a head/rest split: head covers blocks 0-9 (batches 0-4).
                eng.dma_start(out=qt[:, 0:1280], in_=qtb[p][:, 0:1280])
                eng.dma_start(out=kt[:, 0:1664], in_=ktb[p][:, 0:1664])
                eng.dma_start(out=st[:, 0:736], in_=stb[p][:, 0:736])
                eng.dma_start(out=gt, in_=gtb[p])
                eng.dma_start(out=vl[:, 0:12, :], in_=vlb[p][:, 0:12, :])
                eng.dma_start(out=sv[:, 0:5, :, :], in_=svb[p][:, 0:5, :, :])
                eng.dma_start(out=gv, in_=gvb[p])
                eng.dma_start(out=qt[:, 1280:T], in_=qtb[p][:, 1280:T])
                eng.dma_start(out=kt[:, 1664:T + 256], in_=ktb[p][:, 1664:T + 256])
                eng.dma_start(out=st[:, 736:TS + 320], in_=stb[p][:, 736:TS + 320])
                eng.dma_start(out=vl[:, 12:34, :], in_=vlb[p][:, 12:34, :])
                eng.dma_start(out=sv[:, 5:10, :, :], in_=svb[p][:, 5:10, :, :])
            else:
                eng.dma_start(out=qt, in_=qtb[p])
                eng.dma_start(out=kt, in_=ktb[p])
                eng.dma_start(out=st, in_=stb[p])
                eng.dma_start(out=gt, in_=gtb[p])
                eng.dma_start(out=vl, in_=vlb[p])
                eng.dma_start(out=sv, in_=svb[p])
                eng.dma_start(out=gv, in_=gvb[p])

            ob = outp.tile([128, NBLK, D], dt.float32, tag="ob")

            for i in range(NBLK // 2):
                blocks = (2 * i, 2 * i + 1)
                # --- scoresT: per block up to 5 chunks x (128 keys, 128
                # queries), K=64. Fully structural-pad chunks at the sequence
                # edges are dropped: their exp contribution is exactly zero
                # (zero value rows), so skipping them is an identity.
                ents = []
                for bi, b in enumerate(blocks):
                    e = []
                    for j in range(3):
                        if 1 <= b + j <= 32:
                            e.append((kt[:, (b + j) * 128:(b + j + 1) * 128],
                                      vl[:, b + j, :]))
                    if b >= 2:
                        e.append((st[:, b * 32:b * 32 + 128],
                                  sv[:, b // 4, b % 4, :]))
                    if b < 30:
                        w2 = b + 7
                        e.append((st[:, b * 32 + 224:b * 32 + 352],
                                  sv[:, w2 // 4, w2 % 4, :]))
                    ents.append(e)
                na, nb = len(ents[0]), len(ents[1])
                goff = (na + nb) * 128
                edge = (na != 5 or nb != 5)
                ps = ps_pool.tile([128, FPAD], dt.float32, tag="ps")
                # global scores first: both blocks share the [128,128] tile at
                # goff. Both matmuls use row-group A (operands on partitions
                # 0:64) so they serialize (the two col-groups write the same
                # PSUM bank), and the following A0 matmul is also row-group A
                # so it cannot overlap gB's write.
                prev = None
                for bi, b in enumerate(blocks):
                    inst = nc.tensor.matmul(
                        ps[bi * 64:bi * 64 + 64, goff:goff + 128],
                        gt[0:64, 0:64],
                        qt[0:64, b * 128:(b + 1) * 128],
                        start=True, stop=True)
                    if prev is not None:
                        add_dep_helper(inst.ins, prev.ins, sync=False)
                    prev = inst
                if edge:
                    # edge batches: every matmul on row-group A, strictly
                    # serial, so the irregular PSUM bank layout cannot put two
                    # concurrent row-group matmuls in one bank.
                    for bi, b in enumerate(blocks):
                        for k in range(len(ents[bi])):
                            off = ((na if bi else 0) + k) * 128
                            inst = nc.tensor.matmul(
                                ps[:, off:off + 128],
                                ents[bi][k][0][0:64, :],
                                qt[0:64, b * 128:(b + 1) * 128],
                                start=True, stop=True)
                            add_dep_helper(inst.ins, prev.ins, sync=False)
                            prev = inst
                else:
                    # interior: chunk-major (A-seg, B-seg) adjacent matmuls use
                    # different PE row-groups (concurrent) AND land in
                    # different PSUM banks — same-bank concurrency is fatal.
                    for k in range(5):
                        for bi, b in enumerate(blocks):
                            rows = slice(bi * 64, bi * 64 + 64)
                            off = (bi * 5 + k) * 128
                            inst = nc.tensor.matmul(
                                ps[:, off:off + 128],
                                ents[bi][k][0][rows, :],
                                qt[rows, b * 128:(b + 1) * 128],
                                start=True, stop=True)
                            add_dep_helper(inst.ins, prev.ins, sync=False)
                            prev = inst
                # --- probsT = exp(scoresT/8): one ACT instruction per batch
                pb = probs.tile([128, FTOT], dt.bfloat16, tag="pb")
                nc.scalar.activation(pb[:, 0:goff + 128], ps[:, 0:goff + 128],
                                     mybir.ActivationFunctionType.Exp,
                                     scale=0.125)
                # --- ctx + denominator in natural (q, d) layout: stationary
                # is the probsT chunk (128x128 bf16, FWL), moving is the
                # (values | exp(mask)) chunk (128, 65). One PSUM group:
                # cols 0:64 = unnormalized ctx, col 64 = denominator.
                # padded to [2, 256] = exactly one PSUM bank per buffer
                cx = cx_pool.tile([128, 2, 256], dt.float32, tag="cx")
                first_insts = {}
                last_inst = None
                for bi, b in enumerate(blocks):
                    base = na if bi else 0
                    ops = [(pb[:, (base + k) * 128:(base + k + 1) * 128], val)
                           for k, (_, val) in enumerate(ents[bi])]
                    # K=128 (gv half zeroed) so this matmul serializes with its
                    # full-array neighbors — a K=64 version would overlap them
                    # and collide on the cx PSUM bank.
                    ops.append((pb[:, goff:goff + 128], gv[:, bi, :]))
                    for oi, (lhsT, rhs) in enumerate(ops):
                        start = (bi == 0 and oi == 0)
                        stop = (bi == 1 and oi == len(ops) - 1)
                        inst = nc.tensor.matmul(cx[:, bi, 0:D + 1], lhsT, rhs,
                                                start=start, stop=stop)
                        if oi == 0:
                            first_insts[bi] = inst
                        last_inst_prev, last_inst = last_inst, inst
                # keep the single accumulation group well-ordered: the start
                # MM executes first, the stop MM last
                add_dep_helper(first_insts[1].ins, first_insts[0].ins, sync=False)
                add_dep_helper(last_inst.ins, last_inst_prev.ins, sync=False)
                # --- normalize on DVE into the per-pair SBUF accumulator
                rec = outp.tile([128, 2, 1], dt.float32, tag="rec")
                nc.vector.reciprocal(rec, cx[:, :, D:D + 1])
                for bi in range(2):
                    nc.vector.tensor_scalar_mul(ob[:, 2 * i + bi, :],
                                                cx[:, bi, 0:D], rec[:, bi, :])
                if i == NBLK // 4 - 1:
                    # store the first half early so the tail only waits on the
                    # final 512 KiB
                    nc.sync.dma_start(out=oT[p][:, 0:NBLK // 2, :],
                                      in_=ob[:, 0:NBLK // 2, :])
            nc.sync.dma_start(out=oT[p][:, NBLK // 2:NBLK, :],
                              in_=ob[:, NBLK // 2:NBLK, :])

    nc.compile()
    return nc


def _prep_pair(q, k, v, am, sk, sv, sm, gk, gv, gm):
    """Build the device-layout arrays for one (n, h) pair. All inputs fp32
    numpy: q/k/v (T, D); am (T,); sk/sv (TS, D); sm (TS,); gk/gv (G, D);
    gm (G,). Returns dict of bf16 arrays."""
    def dup(x64):
        return np.concatenate([x64, x64], axis=0)

    def expm(mask_vals):
        # exp(mask): 1.0 for zero mask, 0.0 for -inf-ish masks
        with np.errstate(over="ignore", under="ignore"):
            return np.exp(np.minimum(mask_vals, 60.0)).astype(np.float32)

    qt = dup(q.T)

    kt = np.zeros((64, T + 256), np.float32)
    kt[:, 128:128 + T] = k.T
    kt = dup(kt)

    stm = np.zeros((64, TS + 320), np.float32)
    stm[:, 160:160 + TS] = sk.T
    stm = dup(stm)

    gt = np.zeros((64, 128), np.float32)
    gt[:, :G] = gk.T
    gt = dup(gt)

    # value side: row k scaled by exp(mask_k); appended col = exp(mask_k);
    # structural pads stay all-zero.
    em = expm(am)
    vpad = np.zeros((T + 256, D + 1), np.float32)
    vpad[128:128 + T, :D] = v * em[:, None]
    vpad[128:128 + T, D] = em
    vlb = vpad.reshape(34, 128, D + 1).transpose(1, 0, 2)

    esm = expm(sm)
    spad = np.zeros((TS + 320, D + 1), np.float32)
    spad[160:160 + TS, :D] = sv * esm[:, None]
    spad[160:160 + TS, D] = esm
    svb = np.zeros((128, 10, 4, D + 1), np.float32)
    for r in range(4):
        nj = 10 if r < 3 else 9
        for j in range(nj):
            svb[:, j, r] = spad[32 * r + 128 * j: 32 * r + 128 * j + 128]

    egm = expm(gm)
    # slot bi keeps only the partition half holding block bi's global probs;
    # the other half is zero so the K=128 ctx matmul ignores it.
    gvb = np.zeros((128, 2, D + 1), np.float32)
    gvb[:G, 0, :D] = gv * egm[:, None]
    gvb[:G, 0, D] = egm
    gvb[64:64 + G, 1, :] = gvb[:G, 0, :]

    return dict(qtb=qt.astype(BF16), ktb=kt.astype(BF16), stb=stm.astype(BF16),
                gtb=gt.astype(BF16), vlb=vlb.astype(BF16), svb=svb.astype(BF16),
                gvb=gvb.astype(BF16))


def prep_inputs(inputs):
    """Full inputs -> list of per-core in_maps."""
    q = np.asarray(inputs["query_layer"], np.float32)
    k = np.asarray(inputs["key_layer"], np.float32)
    v = np.asarray(inputs["value_layer"], np.float32)
    am = np.asarray(inputs["attention_mask"], np.float32)[:, 0, 0, :]
    sk = np.asarray(inputs["sparse_key"], np.float32)
    sv = np.asarray(inputs["sparse_value"], np.float32)
    sm = np.asarray(inputs["sparse_mask"], np.float32)[:, 0, 0, :]
    gk = np.asarray(inputs["global_key"], np.float32)
    gv = np.asarray(inputs["global_value"], np.float32)
    gm = np.asarray(inputs["global_mask"], np.float32)[:, 0, 0, :]

    in_maps = []
    for c in range(NCORES):
        per_key = {}
        for pp in range(PAIRS):
            pair = c * PAIRS + pp
            n, h = divmod(pair, H)
            d = _prep_pair(q[n, h], k[n, h], v[n, h], am[n],
                           sk[n, h], sv[n, h], sm[n], gk[n, h], gv[n, h], gm[n])
            for name, arr in d.items():
                per_key.setdefault(name, []).append(arr)
        in_maps.append({name: np.stack(arrs) for name, arrs in per_key.items()})
    return in_maps


_prog_cache = {}


def _get_program():
    if "nc" not in _prog_cache:
        _prog_cache["nc"] = build_program()
    return _prog_cache["nc"]


def kernel(**inputs):
    global LAST_RESULTS
    nc = _get_program()
    in_maps = prep_inputs(inputs)
    res = run_bass_kernel_spmd(nc, in_maps, list(range(NCORES)))
    LAST_RESULTS = res
    out = np.empty((N, H, T, D), np.float32)
    for c in range(NCORES):
        oT = res.results[c]["oT"]  # (PAIRS, 128, NBLK, D)
        for pp in range(PAIRS):
            pair = c * PAIRS + pp
            n, h = divmod(pair, H)
            out[n, h] = oT[pp].transpose(1, 0, 2).reshape(T, D)
    return out


# revision 23
# speedup vs baseline: 1.0162x; 1.0162x over previous
"""LSG (local-sparse-global) block attention on 8 trn2 NeuronCores.

Shape/config facts hardcoded from the problem spec:
  n=2 h=12 t=4096 d=64, BLOCK=128, SPARSE_BLOCK=128, SF=4 -> ts=1024, g=64.
Per 128-token query block b the key set is:
  global (64) | sparse W1 [32b-160,32b-32) | sparse W2 [32b+64,32b+192)
  | local [128(b-1), 128(b+2))                              -> 704 real keys.

Sharding: n*h = 24 pairs, 3 per core (data parallel, no cross-core comm).

Device-side design (per pair, 32 blocks, processed as 16 2-block batches):
 - Host ships Q^T / K^T (d-major, 64 rows duplicated to 128 partitions so
   consecutive matmuls can run concurrently in the two PE row-groups, and
   DMA uses all SBUF ports).
 - Masking is folded into the value side: softmax(s+m) = exp(s)*exp(m)
   normalized, so the host scales each key's value row and the appended
   ones-column by exp(mask) (zero at structural pads). Zero-padded keys
   then produce exp(0)=1 junk probs that contribute exactly 0.
 - scoresT (keys on partitions, queries free) per batch: 5 chunks x 128
   keys per block via K=64 matmuls, row-group alternating; the global
   scores of the two blocks share ONE [128,128] tile (partitions 0:64 =
   block A's 64 global keys, 64:128 = block B's) via column-group
   targeting, trimming the exp free size 1536 -> 1408.
 - One Exp ACT per batch over the whole [128,1408] PSUM tile (bf16 out).
 - ctx matmuls: stationary = probsT chunk (128x128 bf16, FWL), moving =
   (values | exp(mask)) chunk (128, 65). One PSUM group per batch
   accumulates ctx (q, d) plus the softmax denominator in column 64.
 - Normalize on DVE into a per-pair SBUF accumulator [128, 32, 64];
   ONE contiguous 1 MiB output DMA per pair (8 KiB per partition) instead
   of per-batch scattered 256 B-descriptor stores; host untransposes.
"""

import numpy as np
import ml_dtypes
from contextlib import ExitStack

import concourse.bacc as bacc
import concourse.bass as bass
import concourse.tile as tile
from concourse import mybir
from concourse.bass_utils import run_bass_kernel_spmd
from concourse.tile import add_dep_helper

N, H, T, D = 2, 12, 4096, 64
TS, G = 1024, 64
NCORES = 8
PAIRS = (N * H) // NCORES  # 3
NBLK = T // 128            # 32
BF16 = ml_dtypes.bfloat16

# free-axis offsets (elements) within the per-batch score tile [128, 1408]
SEG = 640            # per-block segment: 3 local + W1 + W2 chunks
GOFF = 2 * SEG       # shared global tile at 1280..1408
FTOT = GOFF + 128    # 1408
FPAD = 1536          # ps tile padded to exactly 3 PSUM banks so the two
                     # pool buffers never share a bank with the ACT reader

LAST_RESULTS = None  # BassKernelResults of the most recent run (for test.py)


def build_program(pairs=PAIRS):
    dt = mybir.dt
    nc = bacc.Bacc("TRN2", target_bir_lowering=False, debug=False)

    qtb = nc.dram_tensor("qtb", [pairs, 128, T], dt.bfloat16, kind="ExternalInput").ap()
    ktb = nc.dram_tensor("ktb", [pairs, 128, T + 256], dt.bfloat16, kind="ExternalInput").ap()
    stb = nc.dram_tensor("stb", [pairs, 128, TS + 320], dt.bfloat16, kind="ExternalInput").ap()
    gtb = nc.dram_tensor("gtb", [pairs, 128, 128], dt.bfloat16, kind="ExternalInput").ap()
    vlb = nc.dram_tensor("vlb", [pairs, 128, 34, D + 1], dt.bfloat16, kind="ExternalInput").ap()
    svb = nc.dram_tensor("svb", [pairs, 128, 10, 4, D + 1], dt.bfloat16, kind="ExternalInput").ap()
    gvb = nc.dram_tensor("gvb", [pairs, 128, 2, D + 1], dt.bfloat16, kind="ExternalInput").ap()
    # output in partition-major layout: oT[p, q, blk, :] = ctx[blk*128+q, :]
    oT = nc.dram_tensor("oT", [pairs, 128, NBLK, D], dt.float32, kind="ExternalOutput").ap()

    with tile.TileContext(nc) as tc, ExitStack() as ctx:
        big = ctx.enter_context(tc.tile_pool(name="big", bufs=2))
        probs = ctx.enter_context(tc.tile_pool(name="probs", bufs=3))
        outp = ctx.enter_context(tc.tile_pool(name="outp", bufs=2))
        ps_pool = ctx.enter_context(tc.tile_pool(name="psp", bufs=2, space="PSUM"))
        cx_pool = ctx.enter_context(tc.tile_pool(name="cxp", bufs=2, space="PSUM"))

        # warm-up activation at t=0: pulls the ~2.7 us exp ACT_TABLE_LOAD off
        # the critical path so it overlaps the first input DMAs.
        warm = probs.tile([128, 2], dt.bfloat16, tag="warm")
        nc.scalar.memzero(warm)
        nc.scalar.activation(warm, warm, mybir.ActivationFunctionType.Exp)

        for p in range(pairs):
            # single HWDGE ring: FIFO issue order doubles as transfer priority
            # (SDMA round-robin has no usable QoS). Order = what batch 0 needs,
            # then value tensors, then the bulk.
            eng = nc.sync
            qt = big.tile([128, T], dt.bfloat16, tag="qt")
            kt = big.tile([128, T + 256], dt.bfloat16, tag="kt")
            st = big.tile([128, TS + 320], dt.bfloat16, tag="st")
            gt = big.tile([128, 128], dt.bfloat16, tag="gt")
            vl = big.tile([128, 34, D + 1], dt.bfloat16, tag="vl")
            sv = big.tile([128, 10, 4, D + 1], dt.bfloat16, tag="sv")
            gv = big.tile([128, 2, D + 1], dt.bfloat16, tag="gv")
            if p == 0:
                # split the score-side loads so the first batches' slices land
                # early and the exp stream starts immediately.
                # boundaries are window-aligned so no batch's slice straddles
                # a head/rest split: head covers blocks 0-9 (batches 0-4).
                eng.dma_start(out=qt[:, 0:1280], in_=qtb[p][:, 0:1280])
                eng.dma_start(out=kt[:, 0:1664], in_=ktb[p][:, 0:1664])
                eng.dma_start(out=st[:, 0:736], in_=stb[p][:, 0:736])
                eng.dma_start(out=gt, in_=gtb[p])
                eng.dma_start(out=vl[:, 0:12, :], in_=vlb[p][:, 0:12, :])
                eng.dma_start(out=sv[:, 0:5, :, :], in_=svb[p][:, 0:5, :, :])
                eng.dma_start(out=gv, in_=gvb[p])
                eng.dma_start(out=qt[:, 1280:T], in_=qtb[p][:, 1280:T])
                eng.dma_start(out=kt[:, 1664:T + 256], in_=ktb[p][:, 1664:T + 256])
                eng.dma_start(out=st[:, 736:TS + 320], in_=stb[p][:, 736:TS + 320])
                eng.dma_start(out=vl[:, 12:34, :], in_=vlb[p][:, 12:34, :])
                eng.dma_start(out=sv[:, 5:10, :, :], in_=svb[p][:, 5:10, :, :])
            else:
                eng.dma_start(out=qt, in_=qtb[p])
                eng.dma_start(out=kt, in_=ktb[p])
                eng.dma_start(out=st, in_=stb[p])
                eng.dma_start(out=gt, in_=gtb[p])
                eng.dma_start(out=vl, in_=vlb[p])
                eng.dma_start(out=sv, in_=svb[p])
                eng.dma_start(out=gv, in_=gvb[p])

            ob = outp.tile([128, NBLK, D], dt.float32, tag="ob")

            for i in range(NBLK // 2):
                blocks = (2 * i, 2 * i + 1)
                # --- scoresT: per block up to 5 chunks x (128 keys, 128
                # queries), K=64. Fully structural-pad chunks at the sequence
                # edges are dropped: their exp contribution is exactly zero
                # (zero value rows), so skipping them is an identity.
                ents = []
                for bi, b in enumerate(blocks):
                    e = []
                    for j in range(3):
                        if 1 <= b + j <= 32:
                            e.append((kt[:, (b + j) * 128:(b + j + 1) * 128],
                                      vl[:, b + j, :]))
                    if b >= 2:
                        e.append((st[:, b * 32:b * 32 + 128],
                                  sv[:, b // 4, b % 4, :]))
                    if b < 30:
                        w2 = b + 7
                        e.append((st[:, b * 32 + 224:b * 32 + 352],
                                  sv[:, w2 // 4, w2 % 4, :]))
                    ents.append(e)
                na, nb = len(ents[0]), len(ents[1])
                goff = (na + nb) * 128
                edge = (na != 5 or nb != 5)
                ps = ps_pool.tile([128, FPAD], dt.float32, tag="ps")
                # global scores first: both blocks share the [128,128] tile at
                # goff. Both matmuls use row-group A (operands on partitions
                # 0:64) so they serialize (the two col-groups write the same
                # PSUM bank), and the following A0 matmul is also row-group A
                # so it cannot overlap gB's write.
                prev = None
                for bi, b in enumerate(blocks):
                    inst = nc.tensor.matmul(
                        ps[bi * 64:bi * 64 + 64, goff:goff + 128],
                        gt[0:64, 0:64],
                        qt[0:64, b * 128:(b + 1) * 128],
                        start=True, stop=True)
                    if prev is not None:
                        add_dep_helper(inst.ins, prev.ins, sync=False)
                    prev = inst
                if edge:
                    # edge batches: every matmul on row-group A, strictly
                    # serial, so the irregular PSUM bank layout cannot put two
                    # concurrent row-group matmuls in one bank.
                    for bi, b in enumerate(blocks):
                        for k in range(len(ents[bi])):
                            off = ((na if bi else 0) + k) * 128
                            inst = nc.tensor.matmul(
                                ps[:, off:off + 128],
                                ents[bi][k][0][0:64, :],
                                qt[0:64, b * 128:(b + 1) * 128],
                                start=True, stop=True)
                            add_dep_helper(inst.ins, prev.ins, sync=False)
                            prev = inst
                else:
                    # interior: chunk-major (A-seg, B-seg) adjacent matmuls use
                    # different PE row-groups (concurrent) AND land in
                    # different PSUM banks — same-bank concurrency is fatal.
                    for k in range(5):
                        for bi, b in enumerate(blocks):
                            rows = slice(bi * 64, bi * 64 + 64)
                            off = (bi * 5 + k) * 128
                            inst = nc.tensor.matmul(
                                ps[:, off:off + 128],
                                ents[bi][k][0][rows, :],
                                qt[rows, b * 128:(b + 1) * 128],
                                start=True, stop=True)
                            add_dep_helper(inst.ins, prev.ins, sync=False)
                            prev = inst
                # --- probsT = exp(scoresT/8): one ACT instruction per batch
                pb = probs.tile([128, FTOT], dt.bfloat16, tag="pb")
                nc.scalar.activation(pb[:, 0:goff + 128], ps[:, 0:goff + 128],
                                     mybir.ActivationFunctionType.Exp,
                                     scale=0.125)
                # --- ctx + denominator in natural (q, d) layout: stationary
                # is the probsT chunk (128x128 bf16, FWL), moving is the
                # (values | exp(mask)) chunk (128, 65). One PSUM group:
                # cols 0:64 = unnormalized ctx, col 64 = denominator.
                # padded to [2, 256] = exactly one PSUM bank per buffer
                cx = cx_pool.tile([128, 2, 256], dt.float32, tag="cx")
                first_insts = {}
                last_inst = None
                for bi, b in enumerate(blocks):
                    base = na if bi else 0
                    ops = [(pb[:, (base + k) * 128:(base + k + 1) * 128], val)
                           for k, (_, val) in enumerate(ents[bi])]
                    # K=128 (gv half zeroed) so this matmul serializes with its
                    # full-array neighbors — a K=64 version would overlap them
                    # and collide on the cx PSUM bank.
                    ops.append((pb[:, goff:goff + 128], gv[:, bi, :]))
                    for oi, (lhsT, rhs) in enumerate(ops):
                        start = (bi == 0 and oi == 0)
                        stop = (bi == 1 and oi == len(ops) - 1)
                        inst = nc.tensor.matmul(cx[:, bi, 0:D + 1], lhsT, rhs,
                                                start=start, stop=stop)
                        if oi == 0:
                            first_insts[bi] = inst
                        last_inst_prev, last_inst = last_inst, inst
                # keep the single accumulation group well-ordered: the start
                # MM executes first, the stop MM last
                add_dep_helper(first_insts[1].ins, first_insts[0].ins, sync=False)
                add_dep_helper(last_inst.ins, last_inst_prev.ins, sync=False)
                # --- normalize on DVE into the per-pair SBUF accumulator
                rec = outp.tile([128, 2, 1], dt.float32, tag="rec")
                nc.vector.reciprocal(rec, cx[:, :, D:D + 1])
                for bi in range(2):
                    nc.vector.tensor_scalar_mul(ob[:, 2 * i + bi, :],
                                                cx[:, bi, 0:D], rec[:, bi, :])
                if i in (7, 11):
                    # store finished quarters early so the tail only waits on
                    # the final 256 KiB
                    lo, hi = (0, 16) if i == 7 else (16, 24)
                    nc.sync.dma_start(out=oT[p][:, lo:hi, :],
                                      in_=ob[:, lo:hi, :])
            nc.sync.dma_start(out=oT[p][:, 24:NBLK, :], in_=ob[:, 24:NBLK, :])

    nc.compile()
    return nc


def _prep_pair(q, k, v, am, sk, sv, sm, gk, gv, gm):
    """Build the device-layout arrays for one (n, h) pair. All inputs fp32
    numpy: q/k/v (T, D); am (T,); sk/sv (TS, D); sm (TS,); gk/gv (G, D);
    gm (G,). Returns dict of bf16 arrays."""
    def dup(x64):
        return np.concatenate([x64, x64], axis=0)

    def expm(mask_vals):
        # exp(mask): 1.0 for zero mask, 0.0 for -inf-ish masks
        with np.errstate(over="ignore", under="ignore"):
            return np.exp(np.minimum(mask_vals, 60.0)).astype(np.float32)

    qt = dup(q.T)

    kt = np.zeros((64, T + 256), np.float32)
    kt[:, 128:128 + T] = k.T
    kt = dup(kt)

    stm = np.zeros((64, TS + 320), np.float32)
    stm[:, 160:160 + TS] = sk.T
    stm = dup(stm)

    gt = np.zeros((64, 128), np.float32)
    gt[:, :G] = gk.T
    gt = dup(gt)

    # value side: row k scaled by exp(mask_k); appended col = exp(mask_k);
    # structural pads stay all-zero.
    em = expm(am)
    vpad = np.zeros((T + 256, D + 1), np.float32)
    vpad[128:128 + T, :D] = v * em[:, None]
    vpad[128:128 + T, D] = em
    vlb = vpad.reshape(34, 128, D + 1).transpose(1, 0, 2)

    esm = expm(sm)
    spad = np.zeros((TS + 320, D + 1), np.float32)
    spad[160:160 + TS, :D] = sv * esm[:, None]
    spad[160:160 + TS, D] = esm
    svb = np.zeros((128, 10, 4, D + 1), np.float32)
    for r in range(4):
        nj = 10 if r < 3 else 9
        for j in range(nj):
            svb[:, j, r] = spad[32 * r + 128 * j: 32 * r + 128 * j + 128]

    egm = expm(gm)
    # slot bi keeps only the partition half holding block bi's global probs;
    # the other half is zero so the K=128 ctx matmul ignores it.
    gvb = np.zeros((128, 2, D + 1), np.float32)
    gvb[:G, 0, :D] = gv * egm[:, None]
    gvb[:G, 0, D] = egm
    gvb[64:64 + G, 1, :] = gvb[:G, 0, :]

    return dict(qtb=qt.astype(BF16), ktb=kt.astype(BF16), stb=stm.astype(BF16),
                gtb=gt.astype(BF16), vlb=vlb.astype(BF16), svb=svb.astype(BF16),
                gvb=gvb.astype(BF16))


def prep_inputs(inputs):
    """Full inputs -> list of per-core in_maps."""
    q = np.asarray(inputs["query_layer"], np.float32)
    k = np.asarray(inputs["key_layer"], np.float32)
    v = np.asarray(inputs["value_layer"], np.float32)
    am = np.asarray(inputs["attention_mask"], np.float32)[:, 0, 0, :]
    sk = np.asarray(inputs["sparse_key"], np.float32)
    sv = np.asarray(inputs["sparse_value"], np.float32)
    sm = np.asarray(inputs["sparse_mask"], np.float32)[:, 0, 0, :]
    gk = np.asarray(inputs["global_key"], np.float32)
    gv = np.asarray(inputs["global_value"], np.float32)
    gm = np.asarray(inputs["global_mask"], np.float32)[:, 0, 0, :]

    in_maps = []
    for c in range(NCORES):
        per_key = {}
        for pp in range(PAIRS):
            pair = c * PAIRS + pp
            n, h = divmod(pair, H)
            d = _prep_pair(q[n, h], k[n, h], v[n, h], am[n],
                           sk[n, h], sv[n, h], sm[n], gk[n, h], gv[n, h], gm[n])
            for name, arr in d.items():
                per_key.setdefault(name, []).append(arr)
        in_maps.append({name: np.stack(arrs) for name, arrs in per_key.items()})
    return in_maps


_prog_cache = {}


def _get_program():
    if "nc" not in _prog_cache:
        _prog_cache["nc"] = build_program()
    return _prog_cache["nc"]


def kernel(**inputs):
    global LAST_RESULTS
    nc = _get_program()
    in_maps = prep_inputs(inputs)
    res = run_bass_kernel_spmd(nc, in_maps, list(range(NCORES)))
    LAST_RESULTS = res
    out = np.empty((N, H, T, D), np.float32)
    for c in range(NCORES):
        oT = res.results[c]["oT"]  # (PAIRS, 128, NBLK, D)
        for pp in range(PAIRS):
            pair = c * PAIRS + pp
            n, h = divmod(pair, H)
            out[n, h] = oT[pp].transpose(1, 0, 2).reshape(T, D)
    return out
